# revision 1
# baseline (speedup 1.0000x reference)
"""Trainium2 Bass kernel for AccentVarianceAdaptor.

Computation (per batch row):
  pbin = searchsorted(linspace(50,400,256), clip(pitch,50,400), 'left')
  ebin = searchsorted(linspace(0,1,256),  clip(energy,0,1),  'left')
  y    = encoder + ptab[pbin] + etab[ebin]               # [S, H]
  dur  = max(round(duration), 1); cum = cumsum(dur)
  out[t] = y[searchsorted(cum, t, 'right')] * (t < cum[-1])  # [T, H]

Mapping to the hardware (one NeuronCore handles 4 batch rows):
  - table lookup: C[bin, tok] = (boundary[bin] < v[tok]) built with a K=1
    broadcast matmul + tensor_scalar(is_gt); then
    y = base + C_half0.T @ dTab0 + C_half1.T @ dTab1 + encoder,
    where dTab[i] = tab[i+1] - tab[i] (telescoping sum == exact row select).
  - durations: (d + 2^23) - 2^23 rounds half-to-even exactly in f32;
    cumulative sums via triangular matmuls with a PE-transpose supplying the
    inter-chunk offsets.
  - length-regulate: scatter 1.0 at delta[cum[j]] (indirect DMA, one index
    per partition); frame_idx = inclusive-prefix-sum(delta) computed with
    triangular matmuls in a 16-partition layout matching dma_gather's index
    format; then dma_gather pulls y rows (row 512 = zeros covers the ragged
    tail) and plain DMA stores the frames contiguously.
"""

import os
import sys

for _p in ("/opt/trn_rl_repo", "/root/.axon_site/_ro/trn_rl_repo"):
    if os.path.isdir(_p) and _p not in sys.path:
        sys.path.insert(0, _p)

import numpy as np

from concourse import bacc, mybir, tile
from concourse.bass import AP, IndirectOffsetOnAxis, ts
from concourse.bass_utils import run_bass_kernel_spmd

B, S, H = 32, 512, 256
NBINS = 256
T = 4096
NCORES = 8
BPC = B // NCORES  # batches per core
P = 128
NCH = S // P  # token chunks per batch
YROWS = S + 8  # y scratch rows per batch (512 tokens + zero rows)
DELTA_N = T + 8
GCHUNK = 1024  # max indices per dma_gather (SWDGE ring limit)
NGATHER = T // GCHUNK
F32 = mybir.dt.float32
I32 = mybir.dt.int32
I16 = mybir.dt.int16
A = mybir.AluOpType


def _boundaries():
    """Bit-exact copies of the f32 boundaries the jax reference uses."""
    import jax

    with jax.default_device(jax.devices("cpu")[0]):
        import jax.numpy as jnp

        bp = np.asarray(jnp.linspace(50.0, 400.0, NBINS), np.float32)
        be = np.asarray(jnp.linspace(0.0, 1.0, NBINS), np.float32)
    return bp, be


def _host_constants(pitch_table, energy_table):
    bp, be = _boundaries()
    consts = {}
    import ml_dtypes
    for name, tab in (("dpt", pitch_table), ("det", energy_table)):
        d = np.zeros((NBINS, H), np.float32)
        d[:-1] = tab[1:] - tab[:-1]  # f32 arithmetic, row 255 stays 0
        hi = d.astype(ml_dtypes.bfloat16)
        lo = (d - hi.astype(np.float32)).astype(ml_dtypes.bfloat16)
        consts[name + "_hi"] = hi
        consts[name + "_lo"] = lo
    consts["base"] = (pitch_table[0] + energy_table[0]).reshape(1, H)
    consts["bndp"] = bp.reshape(2, P).T.copy()  # [128, 2], col h = b[h*128 + p]
    consts["bnde"] = be.reshape(2, P).T.copy()
    j = np.arange(P, dtype=np.float32)
    consts["lt128"] = (j[:, None] <= j[None, :]).astype(np.float32)  # incl prefix
    consts["slt128"] = (j[:, None] < j[None, :]).astype(np.float32)  # excl prefix
    c4 = np.arange(NCH, dtype=np.float32)
    consts["slt4"] = (c4[:, None] < c4[None, :]).astype(np.float32)
    j16 = np.arange(16, dtype=np.float32)
    consts["lt16"] = (j16[:, None] <= j16[None, :]).astype(np.float32)
    consts["ones1"] = np.ones((1, P), np.float32)
    consts["ones1_16"] = np.ones((1, 16), np.float32)
    consts["onescol16"] = np.ones((16, 1), np.float32)
    consts["onecol"] = np.ones((P, 1), np.float32)
    consts["ident"] = np.eye(P, dtype=np.float32)
    m = np.arange(P)
    consts["rep16"] = (m[None, :] % 16 == np.arange(16)[:, None]).astype(np.float32)
    return consts


def build_nc():
    nc = bacc.Bacc("TRN2", target_bir_lowering=False, debug=False, enable_asserts=False)

    enc_dr = nc.dram_tensor("enc", [BPC, S, H], F32, kind="ExternalInput")
    pit_dr = nc.dram_tensor("pitch", [BPC, S], F32, kind="ExternalInput")
    ene_dr = nc.dram_tensor("energy", [BPC, S], F32, kind="ExternalInput")
    dur_dr = nc.dram_tensor("durt", [BPC, S], F32, kind="ExternalInput")
    BF16 = mybir.dt.bfloat16
    tab_dr = {
        nm: nc.dram_tensor(nm, [NBINS, H], BF16, kind="ExternalInput")
        for nm in ("dpt_hi", "dpt_lo", "det_hi", "det_lo")
    }
    cdr = {
        name: nc.dram_tensor(name, list(arr_shape), F32, kind="ExternalInput")
        for name, arr_shape in (
            ("base", (1, H)),
            ("bndp", (P, 2)),
            ("bnde", (P, 2)),
            ("lt128", (P, P)),
            ("slt128", (P, P)),
            ("slt4", (NCH, NCH)),
            ("lt16", (16, 16)),
            ("ones1", (1, P)),
            ("ones1_16", (1, 16)),
            ("onescol16", (16, 1)),
            ("onecol", (P, 1)),
            ("ident", (P, P)),
            ("rep16", (16, P)),
        )
    }
    out_dr = [
        nc.dram_tensor(f"out{b}", [T, H], F32, kind="ExternalOutput")
        for b in range(BPC)
    ]
    y_dr = nc.dram_tensor("y_scr", [BPC * YROWS, H], F32)
    delta_dr = [nc.dram_tensor(f"delta{b}", [DELTA_N, 1], F32) for b in range(BPC)]

    with tile.TileContext(nc) as tc:
        with (
            tc.tile_pool(name="const", bufs=1) as cp,
            tc.tile_pool(name="work", bufs=4) as wp,
            tc.tile_pool(name="gat", bufs=4) as gp,
            tc.tile_pool(name="idxp", bufs=BPC) as ip,
            tc.tile_pool(name="pbig", bufs=1, space="PSUM") as pb,
            tc.tile_pool(name="peps", bufs=2, space="PSUM") as pe,
            tc.tile_pool(name="psmall", bufs=2, space="PSUM") as psm,
            tc.tile_pool(name="pmicro", bufs=1, space="PSUM") as pmi,
            tc.tile_pool(name="prep", bufs=2, space="PSUM") as prp,
        ):
            # ---- constants ----
            csb = {}
            for name, dr in cdr.items():
                t_ = cp.tile(list(dr.shape), F32, tag=name)
                nc.sync.dma_start(out=t_[:], in_=dr[:])
                csb[name] = t_
            for nm, dr in tab_dr.items():
                t_ = cp.tile([P, 2, H], BF16, tag=nm)
                nc.sync.dma_start(
                    out=t_[:], in_=dr[:].rearrange("(h p) f -> p h f", p=P)
                )
                csb[nm] = t_
            zt = cp.tile([P, H], F32)
            nc.gpsimd.memset(zt[:], 0.0)
            zrow = cp.tile([1, DELTA_N], F32)
            nc.gpsimd.memset(zrow[:], 0.0)

            idx_tiles = {}

            def phase0(b):
                # ---- dur load + frame-delta chain (tiny, unblocks gathers) ----
                dur_raw = wp.tile([P, NCH], F32, tag="draw")
                nc.sync.dma_start(
                    out=dur_raw[:], in_=dur_dr[b].rearrange("(c p) -> p c", p=P)
                )

                # ---- dur = max(round_half_even(durt), 1) ----
                MAGIC = float(1 << 23)
                dr0 = wp.tile([P, NCH], F32, tag="dr0")
                nc.vector.tensor_scalar(out=dr0[:], in0=dur_raw[:], scalar1=MAGIC, scalar2=MAGIC, op0=A.add, op1=A.subtract)
                dur_sb = wp.tile([P, NCH], F32, tag="dur")
                nc.vector.tensor_scalar(out=dur_sb[:], in0=dr0[:], scalar1=1.0, scalar2=None, op0=A.max)

                # ---- inclusive cum over tokens (wrapped j = pc*128 + p) ----
                i1_ps = psm.tile([P, NCH], F32, tag="small")
                nc.tensor.matmul(out=i1_ps[:], lhsT=csb["lt128"][:], rhs=dur_sb[:], start=True, stop=True)
                i1_sb = wp.tile([P, NCH], F32, tag="i1")
                nc.vector.tensor_copy(out=i1_sb[:], in_=i1_ps[:])
                tot_ps = psm.tile([NCH, P], F32, tag="small")
                nc.tensor.transpose(out=tot_ps[:], in_=i1_sb[:], identity=csb["ident"][:])
                tot_sb = wp.tile([NCH, P], F32, tag="tot")
                nc.vector.tensor_copy(out=tot_sb[:], in_=tot_ps[:])
                totb_sb = wp.tile([NCH, P], F32, tag="totb")
                nc.vector.tensor_copy(out=totb_sb[:], in_=tot_sb[:, P - 1 : P].to_broadcast([NCH, P]))
                cum_ps = psm.tile([P, NCH], F32, tag="small")
                nc.tensor.matmul(out=cum_ps[:], lhsT=csb["lt128"][:], rhs=dur_sb[:], start=True, stop=False)
                nc.tensor.matmul(out=cum_ps[:], lhsT=totb_sb[:], rhs=csb["slt4"][:], start=False, stop=True)
                cum_i32 = wp.tile([P, NCH], I32, tag="cumi")
                nc.vector.tensor_copy(out=cum_i32[:], in_=cum_ps[:])

                # ---- delta: zero then scatter 1.0 at cum positions ----
                nc.sync.dma_start(
                    out=delta_dr[b][:].rearrange("v o -> o v"), in_=zrow[:]
                )
                for c in range(NCH):
                    nc.gpsimd.indirect_dma_start(
                        out=delta_dr[b][:],
                        out_offset=IndirectOffsetOnAxis(ap=cum_i32[:, c : c + 1], axis=0),
                        in_=csb["onecol"][:],
                        in_offset=None,
                    )

            def phase1(b):
                # ---- heavy loads + variance-adder compute ----
                enc_sb = wp.tile([P, NCH, H], F32, tag="enc")
                nc.sync.dma_start(
                    out=enc_sb[:], in_=enc_dr[b].rearrange("(c p) f -> p c f", p=P)
                )
                vp_row = wp.tile([1, S], F32, tag="vp")
                ve_row = wp.tile([1, S], F32, tag="ve")
                nc.sync.dma_start(out=vp_row[:], in_=pit_dr[b][None, :])
                nc.sync.dma_start(out=ve_row[:], in_=ene_dr[b][None, :])

                # ---- C matrices: C[bin_p, tok] = (boundary[bin] < v[tok]) ----
                vp_ps = pb.tile([P, S], F32, tag="vps")
                nc.tensor.matmul(out=vp_ps[:], lhsT=csb["ones1"][:], rhs=vp_row[:], start=True, stop=True)
                cp0 = wp.tile([P, S], BF16, tag="cp0")
                cp1 = wp.tile([P, S], BF16, tag="cp1")
                nc.vector.tensor_scalar(out=cp0[:], in0=vp_ps[:], scalar1=csb["bndp"][:, 0:1], scalar2=None, op0=A.is_gt)
                nc.vector.tensor_scalar(out=cp1[:], in0=vp_ps[:], scalar1=csb["bndp"][:, 1:2], scalar2=None, op0=A.is_gt)
                ve_ps = pb.tile([P, S], F32, tag="vps")
                nc.tensor.matmul(out=ve_ps[:], lhsT=csb["ones1"][:], rhs=ve_row[:], start=True, stop=True)
                ce0 = wp.tile([P, S], BF16, tag="ce0")
                ce1 = wp.tile([P, S], BF16, tag="ce1")
                nc.vector.tensor_scalar(out=ce0[:], in0=ve_ps[:], scalar1=csb["bnde"][:, 0:1], scalar2=None, op0=A.is_gt)
                nc.vector.tensor_scalar(out=ce1[:], in0=ve_ps[:], scalar1=csb["bnde"][:, 1:2], scalar2=None, op0=A.is_gt)

                # ---- y = enc + ptab[pbin] + etab[ebin] ----
                y_sb = wp.tile([P, NCH, H], F32, tag="y")
                for c in range(NCH):
                    eps = pe.tile([P, H], F32, tag="eps")
                    first = True
                    for cm, tb in ((cp0, "dpt"), (cp1, "dpt"), (ce0, "det"), (ce1, "det")):
                        hh = 1 if cm in (cp1, ce1) else 0
                        for part in ("_hi", "_lo"):
                            nc.tensor.matmul(out=eps[:], lhsT=cm[:, ts(c, P)], rhs=csb[tb + part][:, hh, :], start=first, stop=False)
                            first = False
                    nc.tensor.matmul(out=eps[:], lhsT=csb["ones1"][:], rhs=csb["base"][:], start=False, stop=True)
                    nc.vector.tensor_tensor(out=y_sb[:, c, :], in0=eps[:], in1=enc_sb[:, c, :], op=A.add)

                # ---- y rows (+ zero padding rows) to HBM scratch ----
                nc.sync.dma_start(
                    out=y_dr[b * YROWS : b * YROWS + S, :].rearrange(
                        "(c p) f -> p c f", p=P
                    ),
                    in_=y_sb[:],
                )
                nc.sync.dma_start(
                    out=y_dr[b * YROWS + S : (b + 1) * YROWS, :], in_=zt[0:8, :]
                )

            def phase2(b):
                # ---- frame_idx = inclusive prefix of delta, 16-wrap layout ----
                d16 = wp.tile([16, T // 16], F32, tag="d16")
                nc.sync.dma_start(
                    out=d16[:],
                    in_=delta_dr[b][0:T, :].rearrange("(s p) o -> p (s o)", p=16),
                )
                # within-column (16 consecutive frames) inclusive prefix
                fi_ps = prp.tile([16, T // 16], F32, tag="rep")
                nc.tensor.matmul(out=fi_ps[:], lhsT=csb["lt16"][:], rhs=d16[:], start=True, stop=True)
                fi_sb = wp.tile([16, T // 16], F32, tag="fis")
                nc.vector.tensor_copy(out=fi_sb[:], in_=fi_ps[:])
                # column sums and their exclusive prefix (two halves of 128 cols)
                cs_sb = wp.tile([P, 2], F32, tag="cs")
                for hf in range(2):
                    cs_ps = pmi.tile([P, 1], F32, tag="micro")
                    nc.tensor.matmul(out=cs_ps[:], lhsT=d16[:, ts(hf, P)], rhs=csb["onescol16"][:], start=True, stop=True)
                    nc.vector.tensor_copy(out=cs_sb[:, hf : hf + 1], in_=cs_ps[:])
                cs0b_sb = wp.tile([P, P], F32, tag="cs0b")
                nc.vector.tensor_copy(out=cs0b_sb[:], in_=cs_sb[:, 0:1].to_broadcast([P, P]))
                cpfx_row = wp.tile([1, T // 16], F32, tag="cpfx")
                for hf in range(2):
                    ep_ps = pmi.tile([P, 1], F32, tag="micro")
                    nc.tensor.matmul(out=ep_ps[:], lhsT=csb["slt128"][:], rhs=cs_sb[:, hf : hf + 1], start=True, stop=hf == 0)
                    if hf == 1:
                        nc.tensor.matmul(out=ep_ps[:], lhsT=cs0b_sb[:], rhs=csb["onecol"][:, 0:1], start=False, stop=True)
                    ep_sb = wp.tile([P, 1], F32, tag="ep")
                    nc.vector.tensor_copy(out=ep_sb[:], in_=ep_ps[:])
                    tr_ps = pmi.tile([1, P], F32, tag="micro")
                    nc.tensor.transpose(out=tr_ps[:], in_=ep_sb[:], identity=csb["ident"][:])
                    nc.vector.tensor_copy(out=cpfx_row[:, ts(hf, P)], in_=tr_ps[:])

                # ---- replicate to 128 partitions + add column offsets; int16 ----
                rep_ps = prp.tile([P, T // 16], F32, tag="rep")
                nc.tensor.matmul(out=rep_ps[:], lhsT=csb["rep16"][:], rhs=fi_sb[:], start=True, stop=False)
                nc.tensor.matmul(out=rep_ps[:], lhsT=csb["ones1"][:], rhs=cpfx_row[:], start=False, stop=True)
                idx16 = ip.tile([P, T // 16], I16, tag=f"idx{b}")
                nc.vector.tensor_copy(out=idx16[:], in_=rep_ps[:])
                idx_tiles[b] = idx16

            def phase3(b):
                # ---- gather frames from y scratch; row 512 = zeros ----
                idx16 = idx_tiles[b]
                ysrc = y_dr[b * YROWS : (b + 1) * YROWS, :]
                for g in range(NGATHER):
                    g_sb = gp.tile([P, GCHUNK // P, H], F32, tag="g")
                    nc.gpsimd.dma_gather(
                        out_ap=g_sb[:],
                        in_ap=ysrc,
                        idxs_ap=idx16[:, g * (GCHUNK // 16) : (g + 1) * (GCHUNK // 16)],
                        num_idxs=GCHUNK,
                        num_idxs_reg=GCHUNK,
                        elem_size=H,
                    )
                    nc.sync.dma_start(
                        out=out_dr[b][g * GCHUNK : (g + 1) * GCHUNK, :].rearrange(
                            "(c p) f -> p c f", p=P
                        ),
                        in_=g_sb[:],
                    )

            for b in range(BPC):
                phase0(b)
            for b in range(BPC):
                phase2(b)
            for b in range(BPC):
                phase1(b)
            for b in range(BPC):
                phase3(b)

    nc.compile()
    return nc


_NC_CACHE = {}


def _get_nc():
    if "nc" not in _NC_CACHE:
        _NC_CACHE["nc"] = build_nc()
    return _NC_CACHE["nc"]


def make_in_maps(inputs):
    enc = np.ascontiguousarray(np.asarray(inputs["encoder_output"], np.float32))
    pit = np.ascontiguousarray(np.asarray(inputs["pitch_target"], np.float32))
    ene = np.ascontiguousarray(np.asarray(inputs["energy_target"], np.float32))
    dur = np.ascontiguousarray(np.asarray(inputs["duration_target"], np.float32))
    ptab = np.asarray(inputs["pitch_table"], np.float32)
    etab = np.asarray(inputs["energy_table"], np.float32)
    consts = _host_constants(ptab, etab)
    in_maps = []
    for c in range(NCORES):
        sl = slice(c * BPC, (c + 1) * BPC)
        m = dict(consts)
        m["enc"] = enc[sl]
        m["pitch"] = pit[sl]
        m["energy"] = ene[sl]
        m["durt"] = dur[sl]
        in_maps.append(m)
    return in_maps


def run(inputs, trace=False):
    nc = _get_nc()
    in_maps = make_in_maps(inputs)
    res = run_bass_kernel_spmd(nc, in_maps, list(range(NCORES)), trace=trace)
    out = np.empty((B, T, H), np.float32)
    for c in range(NCORES):
        for b in range(BPC):
            out[c * BPC + b] = res.results[c][f"out{b}"]
    return out, res


def kernel(**inputs):
    out, _ = run(inputs, trace=False)
    return out



# revision 8
# speedup vs baseline: 1.7042x; 1.7042x over previous
"""Trainium2 Bass kernel for AccentVarianceAdaptor (v2: on-chip one-hot gather).

Computation (per batch row):
  pbin = searchsorted(linspace(50,400,256), clip(pitch,50,400), 'left')
  ebin = searchsorted(linspace(0,1,256),  clip(energy,0,1),  'left')
  y    = encoder + ptab[pbin] + etab[ebin]               # [S, H]
  dur  = max(round(duration), 1); cum = cumsum(dur)
  out[t] = y[searchsorted(cum, t, 'right')] * (t < cum[-1])  # [T, H]

Mapping to the hardware (one NeuronCore handles 4 batch rows):
  - table lookup: C[bin, tok] = (boundary[bin] < v[tok]) built with a K=1
    broadcast matmul + tensor_scalar(is_gt); y = base + sum C_half.T @ dTab
    (telescoping sum == exact row select, bf16 dTab only - tolerance allows).
  - durations: (d + 2^23) - 2^23 rounds half-to-even exactly in f32;
    cumulative sums via triangular matmuls with a PE-transpose supplying the
    inter-chunk offsets.
  - length-regulate entirely on-chip (no HBM scratch, no dma_gather):
    scatter 1.0 at delta[cum[j]]; frame_idx = prefix-sum(delta) via
    triangular matmuls; broadcast idx rows across partitions with K=1
    matmuls; C_onehot[j, t] = is_equal(idx_t, j) in fp16; then
    out[frame_tile] = sum_k C_onehot_k.T @ y_k as fp16 matmuls into PSUM.
    Since dur in [1,8], idx_t in [floor(t/8), min(t,511)], so most
    (token-chunk, frame-tile) blocks are statically zero and skipped
    (74 of 128 matmuls per batch row survive).
"""

import os
import sys

for _p in ("/opt/trn_rl_repo", "/root/.axon_site/_ro/trn_rl_repo"):
    if os.path.isdir(_p) and _p not in sys.path:
        sys.path.insert(0, _p)

import numpy as np

from concourse import bacc, mybir, tile
from concourse.bass import AP, IndirectOffsetOnAxis, ts
from concourse.bass_utils import run_bass_kernel_spmd

B, S, H = 32, 512, 256
NBINS = 256
T = 4096
NCORES = 8
BPC = B // NCORES  # batches per core
P = 128
NCH = S // P  # token chunks per batch (4)
NFG = T // 512  # 512-frame groups per batch (8)
DELTA_N = T + 8
F32 = mybir.dt.float32
F16 = mybir.dt.float16
BF16 = mybir.dt.bfloat16
I32 = mybir.dt.int32
A = mybir.AluOpType
AF = mybir.ActivationFunctionType


def _boundaries():
    """Bit-exact copies of the f32 boundaries the jax reference uses."""
    import jax

    with jax.default_device(jax.devices("cpu")[0]):
        import jax.numpy as jnp

        bp = np.asarray(jnp.linspace(50.0, 400.0, NBINS), np.float32)
        be = np.asarray(jnp.linspace(0.0, 1.0, NBINS), np.float32)
    return bp, be


def _host_constants(pitch_table, energy_table):
    bp, be = _boundaries()
    consts = {}
    import ml_dtypes
    for name, tab in (("dpt", pitch_table), ("det", energy_table)):
        d = np.zeros((NBINS, H), np.float32)
        d[:-1] = tab[1:] - tab[:-1]  # f32 arithmetic, row 255 stays 0
        consts[name + "_hi"] = d.astype(ml_dtypes.bfloat16)
    consts["base"] = (pitch_table[0] + energy_table[0]).reshape(1, H)
    consts["bndp"] = bp.reshape(2, P).T.copy()  # [128, 2], col h = b[h*128 + p]
    consts["bnde"] = be.reshape(2, P).T.copy()
    j = np.arange(P, dtype=np.float32)
    consts["lt128"] = (j[:, None] <= j[None, :]).astype(np.float32)  # incl prefix
    c4 = np.arange(NCH, dtype=np.float32)
    consts["slt4"] = (c4[:, None] < c4[None, :]).astype(np.float32)
    c32 = np.arange(32, dtype=np.float32)
    consts["slt32"] = (c32[:, None] < c32[None, :]).astype(np.float32)
    consts["ones1"] = np.ones((1, P), np.float32)
    consts["ones1h"] = np.ones((1, P), np.float16)
    consts["onecol"] = np.ones((P, 1), np.float32)
    consts["ident"] = np.eye(P, dtype=np.float32)
    tok = (j[:, None] + 128.0 * np.arange(NCH, dtype=np.float32)[None, :])
    consts["tokid"] = tok  # [128, 4], col k = 128k + j
    return consts


def _chunk_range(ft):
    """Token chunks that can feed frame tile ft (dur in [1,8])."""
    return range(ft // 8, min(NCH - 1, ft) + 1)


def build_nc():
    nc = bacc.Bacc("TRN2", target_bir_lowering=False, debug=False, enable_asserts=False)

    enc_dr = nc.dram_tensor("enc", [BPC, S, H], F32, kind="ExternalInput")
    pit_dr = nc.dram_tensor("pitch", [BPC, S], F32, kind="ExternalInput")
    ene_dr = nc.dram_tensor("energy", [BPC, S], F32, kind="ExternalInput")
    dur_dr = nc.dram_tensor("durt", [BPC, S], F32, kind="ExternalInput")
    tab_dr = {
        nm: nc.dram_tensor(nm, [NBINS, H], BF16, kind="ExternalInput")
        for nm in ("dpt_hi", "det_hi")
    }
    cdr = {}
    for name, shape, dt in (
        ("base", (1, H), F32),
        ("bndp", (P, 2), F32),
        ("bnde", (P, 2), F32),
        ("lt128", (P, P), F32),
        ("slt4", (NCH, NCH), F32),
        ("slt32", (32, 32), F32),
        ("ones1", (1, P), F32),
        ("ones1h", (1, P), F16),
        ("onecol", (P, 1), F32),
        ("ident", (P, P), F32),
        ("tokid", (P, NCH), F32),
    ):
        cdr[name] = nc.dram_tensor(name, list(shape), dt, kind="ExternalInput")
    out_dr = [
        nc.dram_tensor(f"out{b}", [T, H], F32, kind="ExternalOutput")
        for b in range(BPC)
    ]
    delta_dr = [nc.dram_tensor(f"delta{b}", [DELTA_N, 1], F32) for b in range(BPC)]

    with tile.TileContext(nc) as tc:
        with (
            tc.tile_pool(name="const", bufs=1) as cp,
            tc.tile_pool(name="work", bufs=2) as wp,
            tc.tile_pool(name="inb", bufs=1) as ib,
            tc.tile_pool(name="ytil", bufs=3) as yp,
            tc.tile_pool(name="cmat", bufs=2) as cpl,
            tc.tile_pool(name="gat", bufs=3) as gp,
            tc.tile_pool(name="pbig", bufs=2, space="PSUM") as pb,
            tc.tile_pool(name="pout", bufs=2, space="PSUM") as po,
            tc.tile_pool(name="peps", bufs=2, space="PSUM") as pe,
            tc.tile_pool(name="psmall", bufs=2, space="PSUM") as psm,
        ):
            # ---- constants ----
            csb = {}
            for name, dr in cdr.items():
                t_ = cp.tile(list(dr.shape), dr.dtype, tag=name)
                nc.sync.dma_start(out=t_[:], in_=dr[:])
                csb[name] = t_
            for nm, dr in tab_dr.items():
                t_ = cp.tile([P, 2, H], BF16, tag=nm)
                nc.sync.dma_start(
                    out=t_[:], in_=dr[:].rearrange("(h p) f -> p h f", p=P)
                )
                csb[nm] = t_
            zrow = cp.tile([1, DELTA_N], F32)
            nc.gpsimd.memset(zrow[:], 0.0)

            # ---- early input prefetch (keeps sync DMA queue busy) ----
            enc_sb, vp_rows, ve_rows, dur_raws = {}, {}, {}, {}
            for b in range(BPC):
                e_ = ib.tile([P, NCH, H], F32, tag=f"enc{b}")
                nc.sync.dma_start(
                    out=e_[:], in_=enc_dr[b].rearrange("(c p) f -> p c f", p=P)
                )
                enc_sb[b] = e_
            for b in range(BPC):
                vp_ = ib.tile([1, S], F32, tag=f"vp{b}")
                ve_ = ib.tile([1, S], F32, tag=f"ve{b}")
                dr_ = ib.tile([P, NCH], F32, tag=f"dw{b}")
                nc.sync.dma_start(out=vp_[:], in_=pit_dr[b][None, :])
                nc.sync.dma_start(out=ve_[:], in_=ene_dr[b][None, :])
                nc.sync.dma_start(
                    out=dr_[:], in_=dur_dr[b].rearrange("(c p) -> p c", p=P)
                )
                vp_rows[b], ve_rows[b], dur_raws[b] = vp_, ve_, dr_

            c_tiles = {}
            y_tiles = {}

            def phase0(b):
                # ---- dur = max(round_half_even(durt), 1) ----
                MAGIC = float(1 << 23)
                dr0 = wp.tile([P, NCH], F32, tag="dr0")
                nc.vector.tensor_scalar(out=dr0[:], in0=dur_raws[b][:], scalar1=MAGIC, scalar2=MAGIC, op0=A.add, op1=A.subtract)
                dur_sb = wp.tile([P, NCH], F32, tag="dur")
                nc.vector.tensor_scalar(out=dur_sb[:], in0=dr0[:], scalar1=1.0, scalar2=None, op0=A.max)

                # ---- inclusive cum over tokens (token s = c*128 + p) ----
                i1_ps = psm.tile([P, NCH], F32, tag="small")
                nc.tensor.matmul(out=i1_ps[:], lhsT=csb["lt128"][:], rhs=dur_sb[:], start=True, stop=True)
                i1_sb = wp.tile([P, NCH], F32, tag="i1")
                nc.vector.tensor_copy(out=i1_sb[:], in_=i1_ps[:])
                tot_ps = psm.tile([NCH, P], F32, tag="small")
                nc.tensor.transpose(out=tot_ps[:], in_=i1_sb[:], identity=csb["ident"][:])
                tot_sb = wp.tile([NCH, P], F32, tag="tot")
                nc.vector.tensor_copy(out=tot_sb[:], in_=tot_ps[:])
                totb_sb = wp.tile([NCH, P], F32, tag="totb")
                nc.vector.tensor_copy(out=totb_sb[:], in_=tot_sb[:, P - 1 : P].to_broadcast([NCH, P]))
                cum_ps = psm.tile([P, NCH], F32, tag="small")
                nc.tensor.matmul(out=cum_ps[:], lhsT=csb["lt128"][:], rhs=dur_sb[:], start=True, stop=False)
                nc.tensor.matmul(out=cum_ps[:], lhsT=totb_sb[:], rhs=csb["slt4"][:], start=False, stop=True)
                cum_i32 = wp.tile([P, NCH], I32, tag="cumi")
                nc.vector.tensor_copy(out=cum_i32[:], in_=cum_ps[:])

                # ---- delta: zero then scatter 1.0 at cum positions ----
                nc.sync.dma_start(
                    out=delta_dr[b][:].rearrange("v o -> o v"), in_=zrow[:]
                )
                for c in range(NCH):
                    nc.gpsimd.indirect_dma_start(
                        out=delta_dr[b][:],
                        out_offset=IndirectOffsetOnAxis(ap=cum_i32[:, c : c + 1], axis=0),
                        in_=csb["onecol"][:],
                        in_offset=None,
                    )

            def phase_idx(b):
                # ---- frame_idx = inclusive prefix of delta, [128, 32] t=c*128+p ----
                d128 = wp.tile([P, 32], F32, tag="d128")
                nc.sync.dma_start(
                    out=d128[:],
                    in_=delta_dr[b][0:T, :].rearrange("(c p) o -> p (c o)", p=P),
                )
                i1_ps = psm.tile([P, 32], F32, tag="small")
                nc.tensor.matmul(out=i1_ps[:], lhsT=csb["lt128"][:], rhs=d128[:], start=True, stop=True)
                i1_sb = wp.tile([P, 32], F32, tag="fi1")
                nc.vector.tensor_copy(out=i1_sb[:], in_=i1_ps[:])
                t_ps = psm.tile([32, P], F32, tag="small")
                nc.tensor.transpose(out=t_ps[:], in_=i1_sb[:], identity=csb["ident"][:])
                totb = wp.tile([32, P], F32, tag="ftot")
                nc.vector.tensor_copy(out=totb[:], in_=t_ps[:, P - 1 : P].to_broadcast([32, P]))
                idx_ps = psm.tile([P, 32], F32, tag="small")
                nc.tensor.matmul(out=idx_ps[:], lhsT=csb["lt128"][:], rhs=d128[:], start=True, stop=False)
                nc.tensor.matmul(out=idx_ps[:], lhsT=totb[:], rhs=csb["slt32"][:], start=False, stop=True)
                idx_sb = wp.tile([P, 32], F32, tag="idxs")
                nc.scalar.activation(out=idx_sb[:], in_=idx_ps[:], func=AF.Copy)
                idxT_ps = psm.tile([32, P], F32, tag="small")
                nc.tensor.transpose(out=idxT_ps[:], in_=idx_sb[:], identity=csb["ident"][:])
                idxT_sb = wp.tile([32, P], F16, tag="idxT")
                nc.scalar.activation(out=idxT_sb[:], in_=idxT_ps[:], func=AF.Copy)
                # reshuffle [32, 128] (t = r*128+p) into a [1, 4096] row on
                # partition 0 so K=1 broadcast matmuls can stream it as rhs
                idx_row = wp.tile([1, T], F16, tag="idxr")
                nc.sync.dma_start(
                    out=idx_row[0:1, :].rearrange("o (r p) -> o r p", p=P),
                    in_=idxT_sb[:],
                )

                # ---- broadcast idx rows to 128 partitions; build one-hot C ----
                for g in range(NFG):
                    bc_ps = pb.tile([P, 512], F32, tag="big")
                    nc.tensor.matmul(
                        out=bc_ps[:], lhsT=csb["ones1h"][:],
                        rhs=idx_row[0:1, ts(g, 512)],
                        start=True, stop=True,
                    )
                    bc_sb = wp.tile([P, 512], F16, tag="bc")
                    nc.scalar.activation(out=bc_sb[:], in_=bc_ps[:], func=AF.Copy)
                    for k in range(g // 2, min(NCH - 1, 4 * g + 3) + 1):
                        ftlo = max(4 * g, k)
                        fthi = min(4 * g + 3, 8 * k + 7)
                        if ftlo > fthi:
                            continue
                        c0 = (ftlo - 4 * g) * P
                        c1 = (fthi - 4 * g + 1) * P
                        ct = cpl.tile([P, 512], F16, tag=f"c{g}_{k}")
                        nc.vector.tensor_scalar(
                            out=ct[:, c0:c1], in0=bc_sb[:, c0:c1],
                            scalar1=csb["tokid"][:, k : k + 1], scalar2=None,
                            op0=A.is_equal,
                        )
                        c_tiles[(b, g, k)] = ct

            def phase_y(b):
                # ---- C matrices: C[bin_p, tok] = (boundary[bin] < v[tok]) ----
                vp_ps = pb.tile([P, S], F32, tag="big")
                nc.tensor.matmul(out=vp_ps[:], lhsT=csb["ones1"][:], rhs=vp_rows[b][:], start=True, stop=True)
                cp0 = wp.tile([P, S], BF16, tag="cp0")
                cp1 = wp.tile([P, S], BF16, tag="cp1")
                nc.vector.tensor_scalar(out=cp0[:], in0=vp_ps[:], scalar1=csb["bndp"][:, 0:1], scalar2=None, op0=A.is_gt)
                nc.vector.tensor_scalar(out=cp1[:], in0=vp_ps[:], scalar1=csb["bndp"][:, 1:2], scalar2=None, op0=A.is_gt)
                ve_ps = pb.tile([P, S], F32, tag="big")
                nc.tensor.matmul(out=ve_ps[:], lhsT=csb["ones1"][:], rhs=ve_rows[b][:], start=True, stop=True)
                ce0 = wp.tile([P, S], BF16, tag="ce0")
                ce1 = wp.tile([P, S], BF16, tag="ce1")
                nc.vector.tensor_scalar(out=ce0[:], in0=ve_ps[:], scalar1=csb["bnde"][:, 0:1], scalar2=None, op0=A.is_gt)
                nc.vector.tensor_scalar(out=ce1[:], in0=ve_ps[:], scalar1=csb["bnde"][:, 1:2], scalar2=None, op0=A.is_gt)

                # ---- y = enc + ptab[pbin] + etab[ebin], fp16 ----
                y_sb = yp.tile([P, NCH, H], F16, tag="y")
                for c in range(NCH):
                    eps = pe.tile([P, H], F32, tag="eps")
                    nc.tensor.matmul(out=eps[:], lhsT=cp0[:, ts(c, P)], rhs=csb["dpt_hi"][:, 0, :], start=True, stop=False)
                    nc.tensor.matmul(out=eps[:], lhsT=cp1[:, ts(c, P)], rhs=csb["dpt_hi"][:, 1, :], start=False, stop=False)
                    nc.tensor.matmul(out=eps[:], lhsT=ce0[:, ts(c, P)], rhs=csb["det_hi"][:, 0, :], start=False, stop=False)
                    nc.tensor.matmul(out=eps[:], lhsT=ce1[:, ts(c, P)], rhs=csb["det_hi"][:, 1, :], start=False, stop=False)
                    nc.tensor.matmul(out=eps[:], lhsT=csb["ones1"][:], rhs=csb["base"][:], start=False, stop=True)
                    nc.vector.tensor_tensor(out=y_sb[:, c, :], in0=eps[:], in1=enc_sb[b][:, c, :], op=A.add)
                y_tiles[b] = y_sb

            def phase_out(b):
                # ---- out[ft*128+p, :] = y[idx, :] via one-hot fp16 matmuls ----
                y_sb = y_tiles[b]
                copy_eng = [nc.vector, nc.scalar]
                for g4 in range(4):  # 1024-frame store groups
                    gbuf = gp.tile([P, 8, H], F32, tag="g")
                    for i in range(4):  # pairs of frame tiles
                        fpair = g4 * 4 + i
                        out_ps = po.tile([P, 512], F32, tag="out")
                        for half in range(2):
                            ft = 2 * fpair + half
                            g = ft // 4
                            coff = (ft % 4) * P
                            ks = list(_chunk_range(ft))
                            for j, k in enumerate(ks):
                                nc.tensor.matmul(
                                    out=out_ps[:, half * H : (half + 1) * H],
                                    lhsT=c_tiles[(b, g, k)][:, coff : coff + P],
                                    rhs=y_sb[:, k, :],
                                    start=(j == 0), stop=(j == len(ks) - 1),
                                )
                        eng = copy_eng[fpair % 2]
                        if eng is nc.scalar:
                            nc.scalar.activation(
                                out=gbuf[:, 2 * i : 2 * i + 2, :], in_=out_ps[:], func=AF.Copy
                            )
                        else:
                            eng.tensor_copy(out=gbuf[:, 2 * i : 2 * i + 2, :], in_=out_ps[:])
                    nc.sync.dma_start(
                        out=out_dr[b][g4 * 1024 : (g4 + 1) * 1024, :].rearrange(
                            "(c p) f -> p c f", p=P
                        ),
                        in_=gbuf[:],
                    )

            for b in range(BPC):
                phase0(b)
            phase_idx(0)
            phase_y(0)
            phase_idx(1)
            phase_y(1)
            phase_out(0)
            phase_idx(2)
            phase_y(2)
            phase_out(1)
            phase_idx(3)
            phase_y(3)
            phase_out(2)
            phase_out(3)

    nc.compile()
    return nc


_NC_CACHE = {}


def _get_nc():
    if "nc" not in _NC_CACHE:
        _NC_CACHE["nc"] = build_nc()
    return _NC_CACHE["nc"]


def make_in_maps(inputs):
    enc = np.ascontiguousarray(np.asarray(inputs["encoder_output"], np.float32))
    pit = np.ascontiguousarray(np.asarray(inputs["pitch_target"], np.float32))
    ene = np.ascontiguousarray(np.asarray(inputs["energy_target"], np.float32))
    dur = np.ascontiguousarray(np.asarray(inputs["duration_target"], np.float32))
    ptab = np.asarray(inputs["pitch_table"], np.float32)
    etab = np.asarray(inputs["energy_table"], np.float32)
    consts = _host_constants(ptab, etab)
    in_maps = []
    for c in range(NCORES):
        sl = slice(c * BPC, (c + 1) * BPC)
        m = dict(consts)
        m["enc"] = enc[sl]
        m["pitch"] = pit[sl]
        m["energy"] = ene[sl]
        m["durt"] = dur[sl]
        in_maps.append(m)
    return in_maps


def run(inputs, trace=False):
    nc = _get_nc()
    in_maps = make_in_maps(inputs)
    res = run_bass_kernel_spmd(nc, in_maps, list(range(NCORES)), trace=trace)
    out = np.empty((B, T, H), np.float32)
    for c in range(NCORES):
        for b in range(BPC):
            out[c * BPC + b] = res.results[c][f"out{b}"]
    return out, res


def kernel(**inputs):
    out, _ = run(inputs, trace=False)
    return out
